# revision 1
# baseline (speedup 1.0000x reference)
"""Trainium2 Bass kernel for nn_KGEdges via low-rank trigonometric factorization.

S[b,i,j] = sum_d w[d] * tanh(h[b,j,d] + c[b,i,d]) + mm[b,i] + mm[b,j]
  with h = x@Wh.T + bh, c = x@Wc.T.

Math: tanh(x) ~= a1*x + sum_m b_m sin(om_m x) (weighted LSQ fit, rel err
~2e-3 under the data distribution), and each sine factorizes over h+c:
sin(om(h+c)) = sin(om h)cos(om c) + cos(om h)sin(om c). The (i,j) plane
then comes from ONE PE contraction over (feature, d):

  S[i,j] = sum_{f,d} Cf[f,d,i] * Hf[f,d,j]
  Hf: {1, h, sin(om_m h), cos(om_m h)}            (2M+2 features)
  Cf: {w*a1*c, w*a1, w*b_m cos(om_m c), w*b_m sin(om_m c)}
  (+ a 2-row mask chunk: [1;mm_i] x [mm_j;1])

This replaces 16.7M tanh/core (the baseline's ACT-bound ~109us floor) with
~1M sin/core. Sin's HW spline is only valid on [-pi,pi]; range reduction is
done in int16 fixed point: one fused DVE tensor_scalar per feature computes
k32 = round(v*(om*65536/2pi) + phase + 2^20) -> i32; the *signed low
halfword* of k32 (i16 stride-2 bitcast view) IS the angle mod 2pi mapped
onto [-32768,32768) ~ [-pi,pi), consumed by one giant Sin pass with
scale=2pi/65536. Data-parallel over batch: 1 core per batch element.
"""

import os
import sys

for _p in ("/opt/trn_rl_repo", "/opt/pypackages"):
    if _p not in sys.path and os.path.isdir(_p):
        sys.path.insert(0, _p)

import numpy as np

from concourse import bass, tile
import concourse.mybir as mybir
from concourse.bass_utils import run_bass_kernel_spmd

BS, SL, ENC, ED = 8, 256, 1024, 256
P = 128
KO = ENC // P      # 8 contraction chunks for projections
DH = ED // P       # 2 d-halves
FD = DH * SL       # 512: free size of one (d, s) plane per partition

# sine fit of tanh on N(0,sqrt2): tanh(x) ~= A1*x + sum b sin(om x)
OM = [0.8457601956781701, 1.7785856286560242, 2.9048194005148917]
BM = [0.4985272230495812, 0.1235633227644961, 0.026858281756312094]
M = len(OM)
A1 = 0.2644975130911238
TWO_PI = 2.0 * np.pi
PH_SCALE = 65536.0 / TWO_PI        # angle -> fixed-point units
ACT_SCALE = float(TWO_PI / 65536.0)
OFF = float(2 ** 20)               # keeps the TS output positive in f32
NSLOT = 2 * M                      # int slots per side (sin & cos per freq)

# f32 tail param columns (per-partition vectors)
T_BH = 0                    # bh by d-half                     (DH cols)
T_WB = T_BH + DH            # w*b_m by (m, dh)                 (M*DH cols)
T_WA1 = T_WB + M * DH       # w*a1 by dh                       (DH cols)
T_TOT = T_WA1 + DH

# packed bf16 input: [xS | tail(f32->bf16 pairs) | WcS | WhS]; the tail rides
# inside the second x DMA chunk so it needs no DMA (and receipt) of its own
F_SEC = KO * SL
OFF_X = 0
OFF_TAIL = F_SEC
OFF_WC = F_SEC + 2 * T_TOT
OFF_WH = OFF_WC + F_SEC
F_PRJ = OFF_WH + F_SEC
SEC_OFF = {0: OFF_X, 1: OFF_WC, 2: OFF_WH}
SEC_X, SEC_WC, SEC_WH = 0, 1, 2

F32 = mybir.dt.float32
F16 = mybir.dt.float16
BF16 = mybir.dt.bfloat16
I32 = mybir.dt.int32
I16 = mybir.dt.int16
AF = mybir.ActivationFunctionType
ALU = mybir.AluOpType

_CACHE: dict = {}

_ENGINE_SEM_PREFIXES = ("Activation", "DVE", "PE", "Pool", "SP", "DMAHW", "DMASW")


def _strip_self_waits(raw: bytes) -> bytes:
    """Remove provably-satisfied self-engine semaphore waits; split residual
    multi-waits on operand-free sync instructions (walrus encodes at most one
    sync wait per instruction)."""
    import json

    m = json.loads(raw)
    for fn in m["functions"]:
        seen: dict = {}
        for blk in fn["blocks"]:
            for ins in blk["instructions"]:
                si = ins.get("sync_info") or {}
                upd = si.get("on_update") or []
                own = {
                    u["id"]
                    for u in upd
                    if u.get("sync_type") == "semaphore"
                    and str(u.get("ant_name", "")).startswith(_ENGINE_SEM_PREFIXES)
                }
                ow = si.get("on_wait") or []
                if len(ow) >= 2:
                    kept = []
                    for w in ow:
                        if (
                            w.get("sync_type") == "semaphore"
                            and w["id"] in own
                            and w.get("wait_mode") == "sem-ge-imm"
                            and w.get("wait_value", 1 << 30)
                            <= seen.get(w["id"], 0)
                        ):
                            continue
                        kept.append(w)
                    si["on_wait"] = kept
                for u in upd:
                    if u.get("sync_type") == "semaphore" and u.get(
                        "update_mode"
                    ) in ("sem-inc", "sem-add-imm"):
                        seen[u["id"]] = seen.get(u["id"], 0) + u.get(
                            "update_value", 1
                        )
        nid = [1 << 20]
        for blk in fn["blocks"]:
            out_insts = []
            for ins in blk["instructions"]:
                si = ins.get("sync_info") or {}
                ow = si.get("on_wait") or []
                if len(ow) >= 2 and not ins.get("ins") and not ins.get("outs"):
                    for w in ow[:-1]:
                        clone = json.loads(json.dumps(ins))
                        clone["sync_info"]["on_wait"] = [w]
                        clone["sync_info"]["on_update"] = []
                        clone["name"] = f"I-{nid[0]}"
                        nid[0] += 1
                        out_insts.append(clone)
                    si["on_wait"] = [ow[-1]]
                out_insts.append(ins)
            blk["instructions"] = out_insts
    return json.dumps(m).encode()


def _build():
    nc = bass.Bass()

    inpb = nc.declare_dram_parameter("inpb", [P, F_PRJ], BF16, isOutput=False)
    maskp = nc.declare_dram_parameter("maskp", [2, 2 * SL], BF16, isOutput=False)
    S_out = nc.declare_dram_parameter("S", [SL, SL], F32, isOutput=True)

    with tile.TileContext(nc) as tc:
        with (
            tc.tile_pool(name="const", bufs=1) as cpool,
            tc.tile_pool(name="pproj", bufs=4, space=bass.MemorySpace.PSUM) as pproj,
            tc.tile_pool(name="pacc", bufs=1, space=bass.MemorySpace.PSUM) as pacc,
            tc.tile_pool(name="pjunk", bufs=1, space=bass.MemorySpace.PSUM) as pjunk,
        ):
            # ---- input DMAs (tail first: it gates DVE/ACT absorbers).
            # Chunks are spread over three DMA queues (sync/scalar HWDGE +
            # gpsimd SWDGE) so transfers AND completion receipts overlap, and
            # ordered so the child projection's operands (x, Wc) land first.
            mask_sb = cpool.tile([2, 2 * SL], BF16, tag="mask")
            inp_sb = cpool.tile([P, F_PRJ], BF16, tag="inp")
            NQ = 2
            QW = F_SEC // NQ

            def inp_chunk(sec, q, extra=0):
                lo = SEC_OFF[sec] + q * QW
                hi = lo + QW + extra
                return (inp_sb[:, lo:hi], inpb[:, lo:hi])

            for eng, (dst, src) in [
                (nc.sync, inp_chunk(SEC_X, 0)),
                (nc.gpsimd, inp_chunk(SEC_WH, 0)),
                (nc.scalar, inp_chunk(SEC_X, 1, extra=2 * T_TOT)),
                (nc.sync, inp_chunk(SEC_WC, 0)),
                (nc.gpsimd, inp_chunk(SEC_WH, 1)),
                (nc.scalar, (mask_sb[:, :], maskp[:, :])),
                (nc.sync, inp_chunk(SEC_WC, 1)),
            ]:
                eng.dma_start(out=dst, in_=src)
            tail_sb = inp_sb[:, OFF_TAIL : OFF_TAIL + 2 * T_TOT].bitcast(F32)

            bh_sb = tail_sb[:, T_BH : T_BH + DH]

            # ---- absorbers: fold each DMA semaphore into consumer engines
            junk = pjunk.tile([1, 16], F32, tag="junk")
            junk_n = [0]

            def absorb_pe(ap):
                k = junk_n[0]
                junk_n[0] += 1
                nc.tensor.matmul(
                    junk[:, k : k + 1], ap, ap,
                    start=True, stop=True, skip_group_check=True,
                )

            junk_dve = cpool.tile([P, 1], F32, tag="junk_dve")
            zero_b = cpool.tile([P, 1], F32, tag="zero_b")
            nc.vector.memset(zero_b[:, :], 0.0)
            ones_f16 = cpool.tile([P, FD], F16, tag="ones")
            nc.vector.memset(ones_f16[:, :], 1.0)
            # ACT: trigger the Sin table load (~1.3us DMA on the scalar
            # ring) only after the x-q0 receipt so it doesn't steal the
            # scalar queue's first completion slots from the Wc chunks
            junk_act = cpool.tile([P, 1], F32, tag="junk_act")
            nc.scalar.copy(junk_act[:, :], inp_sb[:, 0:2].bitcast(F32))
            nc.scalar.copy(junk_act[:, :], tail_sb[:, 0:1])  # tail -> ACT
            nc.scalar.activation(
                junk_act[:, :], zero_b[:, :], AF.Sin, bias=zero_b[:, 0:1],
                scale=ACT_SCALE,
            )

            def proj_sl(sec, ko, lo, hi):
                base = SEC_OFF[sec] + ko * SL
                return inp_sb[:, base + lo : base + hi]

            # ================= per-side feature pipeline =================
            # asymmetric frequency groups: head warms up with 1 freq while
            # its slots build; child ends with 1 freq so only 8 matmuls
            # trail the last Sin pass
            GH_A, GH_B = [0], list(range(1, M))
            GC_A, GC_B = list(range(M - 1)), [M - 1]

            KH = KO // NQ  # ko chunks covered by one DMA chunk

            def proj_quarter(sec_w, tiles, q):
                for dh in range(DH):
                    for ko in range(q * KH, (q + 1) * KH):
                        nc.tensor.matmul(
                            tiles[dh][:, :],
                            proj_sl(sec_w, ko, dh * P, (dh + 1) * P),
                            proj_sl(SEC_X, ko, 0, SL),
                            start=(ko == 0),
                            stop=(ko == KO - 1),
                        )


            ps_c0 = pproj.tile([P, SL], F32, tag="proj")
            ps_c1 = pproj.tile([P, SL], F32, tag="proj")
            ps_h0 = pproj.tile([P, SL], F32, tag="proj")
            ps_h1 = pproj.tile([P, SL], F32, tag="proj")
            ps_c, ps_h = [ps_c0, ps_c1], [ps_h0, ps_h1]

            # head completes first; child projects under the head ACT work
            absorb_pe(inp_sb[:, OFF_X : OFF_X + 1])
            absorb_pe(inp_sb[:, OFF_WH : OFF_WH + 1])
            proj_quarter(SEC_WH, ps_h, 0)
            absorb_pe(inp_sb[:, OFF_X + QW : OFF_X + QW + 1])
            absorb_pe(inp_sb[:, OFF_WH + QW : OFF_WH + QW + 1])
            proj_quarter(SEC_WH, ps_h, 1)
            absorb_pe(inp_sb[:, OFF_WC : OFF_WC + 1])
            proj_quarter(SEC_WC, ps_c, 0)
            absorb_pe(inp_sb[:, OFF_WC + QW : OFF_WC + QW + 1])
            proj_quarter(SEC_WC, ps_c, 1)
            absorb_pe(mask_sb[:, 0:1])

            def evict_dh(ps, bias_col, v_sb, dh):
                dst = v_sb[:, dh * SL : (dh + 1) * SL]
                if bias_col is not None:
                    # ACT is idle before its first Sin pass; evicting there
                    # unblocks the DVE slot chain ~0.5us earlier
                    nc.scalar.activation(
                        dst, ps[dh][:, :], AF.Identity,
                        bias=bias_col[:, dh : dh + 1],
                    )
                else:
                    nc.vector.tensor_copy(dst, ps[dh][:, :])

            def slots_dh(ints, v_sb, ms, dh):
                sl_ = slice(dh * SL, (dh + 1) * SL)
                for m in ms:
                    for f in range(2):  # 0: sin slot, 1: cos slot (+pi/2)
                        ph = 0.0 if f == 0 else (np.pi / 2)
                        nc.vector.tensor_scalar(
                            out=ints[:, 2 * m + f, sl_],
                            in0=v_sb[:, sl_],
                            scalar1=float(OM[m] * PH_SCALE),
                            scalar2=float(ph * PH_SCALE + OFF),
                            op0=ALU.mult,
                            op1=ALU.add,
                        )

            def evict_and_slots(ps, bias_col, v_sb, ints, ms):
                for dh in range(DH):
                    evict_dh(ps, bias_col, v_sb, dh)
                    slots_dh(ints, v_sb, ms, dh)

            def slots(ints, v_sb, ms):
                for dh in range(DH):
                    slots_dh(ints, v_sb, ms, dh)

            def giant_sin(ints, feats, ms):
                s0, s1 = 2 * ms[0], 2 * ms[-1] + 2
                nc.scalar.activation(
                    feats[:, s0:s1, :],
                    ints[:, s0:s1, :].bitcast(I16)
                    .rearrange("p s (n two) -> p s n two", two=2)[:, :, :, 0],
                    AF.Sin,
                    bias=zero_b[:, 0:1],
                    scale=ACT_SCALE,
                )

            c_ints = cpool.tile([P, NSLOT, FD], I32, tag="int_c")
            h_ints = cpool.tile([P, NSLOT, FD], I32, tag="int_h")
            c_feats = cpool.tile([P, NSLOT, FD], F16, tag="feat_c")
            h_feats = cpool.tile([P, NSLOT, FD], F16, tag="feat_h")
            csc = cpool.tile([P, NSLOT, FD], F16, tag="csc")
            h_f16 = cpool.tile([P, FD], F16, tag="h16")
            c0 = cpool.tile([P, FD], F16, tag="c0")   # w*a1*c
            c1 = cpool.tile([P, FD], F16, tag="c1")   # w*a1 (constant)

            def fold_hsc(ms):
                # hsc pairs with raw C features: sinC <-> w*b*cosH etc.
                for m in ms:
                    for f in range(2):
                        src = h_feats[:, 2 * m + (1 - f), :]
                        for dh in range(DH):
                            nc.vector.tensor_scalar_mul(
                                csc[:, 2 * m + f, dh * SL : (dh + 1) * SL],
                                src[:, dh * SL : (dh + 1) * SL],
                                tail_sb[:, T_WB + m * DH + dh :
                                        T_WB + m * DH + dh + 1],
                            )

            # DVE program in consumer-priority order; ACT giants interleave
            c_sb = cpool.tile([P, FD], F32, tag="v_c")
            h_sb = cpool.tile([P, FD], F32, tag="v_h")
            nc.vector.tensor_copy(junk_dve[:, :], tail_sb[:, 0:1])  # tail->DVE
            evict_and_slots(ps_h, bh_sb, h_sb, h_ints, GH_A)
            giant_sin(h_ints, h_feats, GH_A)     # ACT 1
            slots(h_ints, h_sb, GH_B)
            giant_sin(h_ints, h_feats, GH_B)     # ACT 2
            for dh in range(DH):
                wa1 = tail_sb[:, T_WA1 + dh : T_WA1 + dh + 1]
                sl_ = slice(dh * SL, (dh + 1) * SL)
                nc.vector.tensor_scalar_mul(h_f16[:, sl_], h_sb[:, sl_], wa1)
                nc.vector.tensor_scalar_mul(c1[:, sl_], ones_f16[:, sl_], wa1)
            evict_and_slots(ps_c, None, c_sb, c_ints, GC_A)
            nc.vector.tensor_copy(c0[:, :], c_sb[:, :])
            giant_sin(c_ints, c_feats, GC_A)     # ACT 3
            slots(c_ints, c_sb, GC_B)
            giant_sin(c_ints, c_feats, GC_B)     # ACT 4
            fold_hsc(GC_A)
            fold_hsc(GC_B)

            # ---- the big contraction: S[i,j] += Cf^T @ Hf per (feature, dh)
            acc0 = pacc.tile([P, SL], F32, tag="acc0")
            acc1 = pacc.tile([P, SL], F32, tag="acc1")
            acc = [acc0, acc1]

            def trig_chunks(ms):
                out = []
                for m in ms:
                    for f in range(2):
                        for dh in range(DH):
                            sl_ = slice(dh * SL, (dh + 1) * SL)
                            out.append((c_feats[:, 2 * m + f, sl_],
                                        csc[:, 2 * m + f, sl_]))
                return out

            # lin pairs: ones_C <-> w*a1*h (h_f16), c (c0) <-> w*a1 (c1)
            lin_chunks = []
            for dh in range(DH):
                sl_ = slice(dh * SL, (dh + 1) * SL)
                lin_chunks.append((ones_f16[:, sl_], h_f16[:, sl_]))
                lin_chunks.append((c0[:, sl_], c1[:, sl_]))

            def mm(ih, lhsT, rhs, start, stop):
                nc.tensor.matmul(
                    acc[ih][:, :], lhsT[:, ih * P : (ih + 1) * P], rhs,
                    start=start, stop=stop,
                )

            # group 1: linear chunks (ready once projections evicted)
            absorb_pe(c0[:, 0:1])
            first = True
            for lhsT, rhs in lin_chunks:
                for ih in range(2):
                    mm(ih, lhsT, rhs, first, False)
                first = False
            # mask chunk (tiny)
            absorb_pe(mask_sb[:, 0:1])
            for ih in range(2):
                nc.tensor.matmul(
                    acc[ih][:, :],
                    mask_sb[:, SL + ih * P : SL + (ih + 1) * P],
                    mask_sb[:, 0:SL],
                    start=False, stop=False,
                )
            # group 2: first-wave trig chunks (after ACT 3 + H-side folds)
            absorb_pe(c_feats[:, 2 * GC_A[-1] + 1, 0:1])
            absorb_pe(csc[:, 2 * GC_A[-1] + 1, 0:1])
            for lhsT, rhs in trig_chunks(GC_A):
                for ih in range(2):
                    mm(ih, lhsT, rhs, False, False)
            # group 3: last trig chunks; finish ih0 first so its epilogue
            # and output DMA overlap ih1's tail
            absorb_pe(c_feats[:, NSLOT - 1, 0:1])
            absorb_pe(csc[:, NSLOT - 1, 0:1])
            tcB = trig_chunks(GC_B)
            s_t = cpool.tile([P, 2, SL], F32, tag="sout")
            for ih in range(2):
                for ci, (lhsT, rhs) in enumerate(tcB):
                    mm(ih, lhsT, rhs, False, ci == len(tcB) - 1)
                nc.vector.tensor_copy(s_t[:, ih, :], acc[ih][:, :])
                (nc.sync if ih == 0 else nc.scalar).dma_start(
                    out=S_out[ih * P : (ih + 1) * P, :], in_=s_t[:, ih, :]
                )

    _orig = nc.to_json_bytes
    nc.to_json_bytes = lambda: _strip_self_waits(_orig())
    return nc


def _prep_in_maps(inputs):
    import ml_dtypes

    bf16 = ml_dtypes.bfloat16
    x = np.ascontiguousarray(np.asarray(inputs["encoded_text"], dtype=np.float32))
    mask = np.asarray(inputs["mask"])
    Wh = np.asarray(inputs["Wh"], dtype=np.float32)
    bh = np.asarray(inputs["bh"], dtype=np.float32)
    Wc = np.asarray(inputs["Wc"], dtype=np.float32)
    w_out = np.asarray(inputs["w_out"], dtype=np.float32)

    def pack_w(W):  # (ED, ENC) -> (P, KO*ED) partition-major, ko-major
        return np.ascontiguousarray(
            W.T.reshape(KO, P, ED).transpose(1, 0, 2).reshape(P, F_SEC)
        ).astype(bf16)

    WhS, WcS = pack_w(Wh), pack_w(Wc)
    mm = ((1.0 - mask.astype(np.float32)) * -1.0e8).astype(np.float32)  # (BS, SL)
    wdh = w_out.reshape(DH, P).T              # (P, DH): w by (dlo, dh)

    tailv = np.zeros((P, T_TOT), dtype=np.float32)
    tailv[:, T_BH : T_BH + DH] = bh.reshape(DH, P).T
    for m in range(M):
        for dh in range(DH):
            tailv[:, T_WB + m * DH + dh] = wdh[:, dh] * BM[m]
    for dh in range(DH):
        tailv[:, T_WA1 + dh] = wdh[:, dh] * A1
    tail_bf = np.ascontiguousarray(tailv).view(bf16)  # (P, 2*T_TOT) raw bytes

    in_maps = []
    for b in range(BS):
        xS = np.ascontiguousarray(
            x[b].T.reshape(KO, P, SL).transpose(1, 0, 2).reshape(P, F_SEC)
        ).astype(bf16)
        packed = np.empty((P, F_PRJ), dtype=bf16)
        packed[:, OFF_X : OFF_X + F_SEC] = xS
        packed[:, OFF_TAIL : OFF_TAIL + 2 * T_TOT] = tail_bf
        packed[:, OFF_WC : OFF_WC + F_SEC] = WcS
        packed[:, OFF_WH : OFF_WH + F_SEC] = WhS
        maskv = np.zeros((2, 2 * SL), dtype=np.float32)
        maskv[0, 0:SL] = mm[b]          # rhs row0: mm_j
        maskv[1, 0:SL] = 1.0            # rhs row1: ones
        maskv[0, SL:] = 1.0             # lhsT row0: ones (pairs with mm_j)
        maskv[1, SL:] = mm[b]           # lhsT row1: mm_i
        in_maps.append(dict(inpb=packed, maskp=maskv.astype(bf16)))
    return in_maps


def run(inputs, trace=False, **kw):
    if "nc" not in _CACHE:
        _CACHE["nc"] = _build()
    nc = _CACHE["nc"]
    in_maps = _prep_in_maps(inputs)
    res = run_bass_kernel_spmd(nc, in_maps, list(range(BS)), trace=trace, **kw)
    out = np.stack([np.asarray(res.results[b]["S"]) for b in range(BS)], axis=0)
    return out.astype(np.float32, copy=False), res


def kernel(**inputs):
    return run(inputs)[0]

